# revision 10
# baseline (speedup 1.0000x reference)
"""Masked attention kernel for Trainium2, data-parallel over 8 NeuronCores.

Problem: out[q,b,:] = softmax-ish(LN(query Wq^T+bq) @ LN(key Wk^T+bk)^T / sqrt(H),
masked by query_mask & key_mask, with the reference's idiosyncratic
exp(s - 2*rowmax) / (sum + 0.001) normalization) @ value.

Key observations exploited:
 - The reference fills masked scores with the GLOBAL min before the row max.
   Every unmasked score >= global min, so the row max equals the max over
   unmasked entries whenever one exists; fully-masked rows output exactly 0.
   Hence zero cross-batch communication: B=8 batches map 1:1 onto 8 cores.
 - Masked-out query rows produce zero output rows; masked-out keys contribute
   nothing.  Both masks are ~50% dense, so each core computes attention only
   over compacted (host-gathered) rows, padded to a fixed size.
 - exp(s - 2m)/(sum + 0.001) == exp(s)/(sum' + 0.001*exp(2m)), and
   exp(2m) == (rowmax of exp(s))^2.  So we take exp with no shift at all
   (scaled scores are O(5), safely in range) and correct the denominator with
   0.001*emax^2 - npad (each padded key column contributes exactly exp(0)=1).
   This kills the entire PSUM row-max reduction pass.

Host side: compact/pad/transpose per batch (cheap numpy), run the SPMD NEFF,
scatter results back into the full [Q,B,H] output.
"""

import numpy as np
import ml_dtypes

import concourse.bacc as bacc
import concourse.bass as bass
import concourse.tile as tile
from concourse import mybir, masks
from concourse.bass_utils import run_bass_kernel_spmd

F32 = mybir.dt.float32
BF16 = mybir.dt.bfloat16
AX = mybir.AxisListType.X
AF = mybir.ActivationFunctionType
ALU = mybir.AluOpType

H = 512
HC = H // 128          # contraction chunks over the hidden dim
NCORES = 8
RSQRT_H = 1.0 / float(np.sqrt(np.float32(H)))
EPS = 1e-5

_cache = {}
last_results = None


def _s_chunks(pad):
    """Split the key axis into <=512-wide chunks, one PSUM bank slot each."""
    nb = -(-pad // 512)
    base, rem = divmod(pad, nb)
    return [base + (1 if i < rem else 0) for i in range(nb)]


def _build(pad, biasq, biask, affq, affk):
    nt = pad // 128
    sch = _s_chunks(pad)
    nsb = len(sch)

    nc = bacc.Bacc(None, target_bir_lowering=False, debug=False, enable_asserts=False)

    xqT_d = nc.declare_dram_parameter("xqT", [H, pad], BF16, isOutput=False)
    xkT_d = nc.declare_dram_parameter("xkT", [H, pad], BF16, isOutput=False)
    v_d = nc.declare_dram_parameter("v", [pad, H], BF16, isOutput=False)
    npad_d = nc.declare_dram_parameter("npad", [1, 1], F32, isOutput=False)
    km_d = None
    if biask or affk:
        km_d = nc.declare_dram_parameter("km01", [pad, 1], F32, isOutput=False)
    wqT_d = nc.declare_dram_parameter("WqT", [H, H], BF16, isOutput=False)
    wkT_d = nc.declare_dram_parameter("WkT", [H, H], BF16, isOutput=False)
    extras_d = {}
    if biasq:
        extras_d["bq"] = nc.declare_dram_parameter("bq", [1, H], F32, isOutput=False)
    if biask:
        extras_d["bk"] = nc.declare_dram_parameter("bk", [1, H], F32, isOutput=False)
    if affq:
        extras_d["gq"] = nc.declare_dram_parameter("gq", [1, H], F32, isOutput=False)
        extras_d["betaq"] = nc.declare_dram_parameter("betaq", [1, H], F32, isOutput=False)
    if affk:
        extras_d["gk"] = nc.declare_dram_parameter("gk", [1, H], F32, isOutput=False)
        extras_d["betak"] = nc.declare_dram_parameter("betak", [1, H], F32, isOutput=False)
    out_d = nc.declare_dram_parameter("out", [pad, H], F32, isOutput=True)

    with tile.TileContext(nc) as tc:
        with (
            tc.tile_pool(name="persist", bufs=1) as persist,
            tc.tile_pool(name="small", bufs=6) as small,
            tc.tile_pool(name="lnt", bufs=4) as lnt,
            tc.tile_pool(name="ework", bufs=2) as ework,
            tc.tile_pool(name="ptw", bufs=3) as ptw,
            tc.tile_pool(name="osb", bufs=2) as osbp,
        ):
            ident16 = persist.tile([128, 128], BF16)
            masks.make_identity(nc, ident16[:])
            c001_t = persist.tile([128, 1], F32)
            nc.vector.memset(c001_t[:], 0.001)
            eps_t = persist.tile([128, 1], F32)
            nc.vector.memset(eps_t[:], EPS)

            # resident inputs; split across two DMA queues for parallel load
            xqT_sb = persist.tile([128, HC, pad], BF16)
            nc.sync.dma_start(out=xqT_sb[:], in_=xqT_d[:, :].rearrange("(c p) t -> p c t", p=128))
            wq_sb = persist.tile([128, HC, H], BF16)
            nc.sync.dma_start(out=wq_sb[:], in_=wqT_d[:, :].rearrange("(c p) i -> p c i", p=128))
            xkT_sb = persist.tile([128, HC, pad], BF16)
            nc.scalar.dma_start(out=xkT_sb[:], in_=xkT_d[:, :].rearrange("(c p) t -> p c t", p=128))
            wk_sb = persist.tile([128, HC, H], BF16)
            nc.scalar.dma_start(out=wk_sb[:], in_=wkT_d[:, :].rearrange("(c p) i -> p c i", p=128))
            v_sb = persist.tile([128, nt, H], BF16)
            nc.scalar.dma_start(out=v_sb[:], in_=v_d[:, :].rearrange("(n p) h -> p n h", p=128))
            npad_t = persist.tile([128, 1], F32)
            np_src = npad_d[:, :]
            np_src = bass.AP(tensor=np_src.tensor, offset=np_src.offset, ap=[[0, 128], [1, 1]])
            nc.scalar.dma_start(out=npad_t[:], in_=np_src)
            kmA_sb = None
            if km_d is not None:
                kmA_sb = persist.tile([128, nt], F32)
                nc.scalar.dma_start(out=kmA_sb[:], in_=km_d[:, :].rearrange("(n p) o -> p (n o)", p=128))
            bcast = {}
            for name, dram in extras_d.items():
                t = persist.tile([128, H], F32, tag=f"bc_{name}")
                src = dram[:, :]
                src = bass.AP(tensor=src.tensor, offset=src.offset, ap=[[0, 128]] + [src.ap[-1]])
                nc.scalar.dma_start(out=t[:], in_=src)
                bcast[name] = t

            qT_sb = persist.tile([128, HC, pad], BF16)
            kT_sb = persist.tile([128, HC, pad], BF16)

            # ---- phase A: project + layernorm + transpose (SW-pipelined) --------
            with tc.tile_pool(name="psA", bufs=4, space="PSUM") as psA:
                jobs = [(s, t) for s in range(2) for t in range(nt)]
                ps_of = {}

                def proj(i):
                    s, t = jobs[i]
                    x_sb = (xqT_sb, xkT_sb)[s]
                    w_sb = (wq_sb, wk_sb)[s]
                    ps = psA.tile([128, H], F32, tag="proj")
                    ps_of[i] = ps
                    for c in range(HC):
                        nc.tensor.matmul(ps[:], x_sb[:, c, t * 128:(t + 1) * 128],
                                         w_sb[:, c, :], start=(c == 0), stop=(c == HC - 1))

                def ln_and_tp(i):
                    s, t = jobs[i]
                    dst = (qT_sb, kT_sb)[s]
                    use_bias = (biasq, biask)[s]
                    use_aff = (affq, affk)[s]
                    ps = ps_of.pop(i)
                    if use_bias:
                        nc.vector.tensor_add(ps[:], ps[:], bcast[("bq", "bk")[s]][:])
                    stats = small.tile([128, 6], F32, tag="stats")
                    nc.vector.bn_stats(out=stats[:], in_=ps[:])
                    mv = small.tile([128, 2], F32, tag="mv")
                    nc.vector.bn_aggr(out=mv[:], in_=stats[:])
                    sd = small.tile([128, 1], F32, tag="sd")
                    nc.scalar.activation(out=sd[:], in_=mv[:, 1:2], func=AF.Sqrt,
                                         bias=eps_t[:], scale=1.0)
                    rstd = small.tile([128, 1], F32, tag="rstd")
                    nc.vector.reciprocal(out=rstd[:], in_=sd[:])
                    nmr = small.tile([128, 1], F32, tag="nmr")
                    nc.vector.tensor_mul(nmr[:], mv[:, 0:1], rstd[:])
                    nc.vector.tensor_scalar_mul(nmr[:], nmr[:], -1.0)
                    ln = lnt.tile([128, H], BF16, tag="ln")
                    if use_aff:
                        ln32 = lnt.tile([128, H], F32, tag="ln32")
                        nc.scalar.activation(out=ln32[:], in_=ps[:], func=AF.Identity,
                                             bias=nmr[:], scale=rstd[:])
                        nc.vector.tensor_mul(ln32[:], ln32[:], bcast[("gq", "gk")[s]][:])
                        nc.vector.tensor_add(ln[:], ln32[:], bcast[("betaq", "betak")[s]][:])
                    elif i % 2 == 0:
                        nc.scalar.activation(out=ln[:], in_=ps[:], func=AF.Identity,
                                             bias=nmr[:], scale=rstd[:])
                    else:
                        nc.vector.tensor_scalar(out=ln[:], in0=ps[:], scalar1=rstd[:],
                                                scalar2=nmr[:], op0=ALU.mult, op1=ALU.add)
                    if s == 1 and kmA_sb is not None:
                        nc.vector.tensor_scalar_mul(ln[:], ln[:], kmA_sb[:, t:t + 1])
                    dslice = dst[:, :, t * 128:(t + 1) * 128]
                    eng = nc.sync if i % 2 == 0 else nc.scalar
                    eng.dma_start_transpose(dslice, ln[:])

                DEPTH = 3
                for i in range(min(DEPTH, len(jobs))):
                    proj(i)
                for i in range(len(jobs)):
                    if i + DEPTH < len(jobs):
                        proj(i + DEPTH)
                    ln_and_tp(i)

            # ---- phase B: attention (SW-pipelined) ------------------------------
            with (
                tc.tile_pool(name="psS", bufs=2, space="PSUM") as psS,
                tc.tile_pool(name="psC", bufs=2, space="PSUM") as psC,
            ):
                S_of = {}

                def qk(t):
                    S = psS.tile([128, nsb, 512], F32, tag="S")
                    S_of[t] = S
                    for c in range(HC):
                        off = 0
                        for j, w in enumerate(sch):
                            nc.tensor.matmul(S[:, j, 0:w],
                                             qT_sb[:, c, t * 128:(t + 1) * 128],
                                             kT_sb[:, c, off:off + w],
                                             start=(c == 0), stop=(c == HC - 1))
                            off += w

                def attend(t):
                    S = S_of.pop(t)
                    e = ework.tile([128, pad], BF16, tag="e")
                    acc3 = small.tile([128, nsb], F32, tag="acc3")
                    ptT = ptw.tile([128, nt, 128], BF16, tag="pt")
                    off = 0
                    for j, w in enumerate(sch):
                        nc.scalar.activation(out=e[:, off:off + w], in_=S[:, j, 0:w],
                                             func=AF.Exp, bias=0.0, scale=RSQRT_H,
                                             accum_out=acc3[:, j:j + 1])
                        eng = nc.sync if j % 2 == 0 else nc.scalar
                        eng.dma_start_transpose(ptT[:, off // 128:(off + w) // 128, :],
                                                e[:, off:off + w])
                        off += w
                    # denom = sum(e) - npad + 0.001 * rowmax(e)^2
                    emax = small.tile([128, 1], F32, tag="emax")
                    nc.vector.reduce_max(emax[:], e[:], axis=AX)
                    dsum = small.tile([128, 1], F32, tag="dsum")
                    nc.vector.reduce_sum(dsum[:], acc3[:], axis=AX)
                    nc.vector.tensor_mul(emax[:], emax[:], emax[:])
                    nc.vector.tensor_scalar(out=emax[:], in0=emax[:], scalar1=0.001,
                                            scalar2=npad_t[:], op0=ALU.mult, op1=ALU.subtract)
                    nc.vector.tensor_add(dsum[:], dsum[:], emax[:])
                    r = small.tile([128, 1], F32, tag="r")
                    nc.vector.reciprocal(out=r[:], in_=dsum[:])

                    C = psC.tile([128, H], F32, tag="C")
                    for j in range(nt):
                        nc.tensor.matmul(C[:], ptT[:, j, :], v_sb[:, j, :],
                                         start=(j == 0), stop=(j == nt - 1))
                    o = osbp.tile([128, H], F32, tag="o")
                    nc.vector.tensor_scalar_mul(o[:], C[:], r[:])
                    nc.sync.dma_start(out=out_d[t * 128:(t + 1) * 128, :], in_=o[:])

                qk(0)
                for t in range(nt):
                    if t + 1 < nt:
                        qk(t + 1)
                    attend(t)

    nc.compile()
    return nc


def _get_nc(pad, biasq, biask, affq, affk):
    key = (pad, biasq, biask, affq, affk)
    if key not in _cache:
        _cache[key] = _build(*key)
    return _cache[key]


def kernel(query, key_in, value, query_mask, key_mask,
           Wq, bq, gq, betaq, Wk, bk, gk, betak):
    query = np.asarray(query, np.float32)
    key_in = np.asarray(key_in, np.float32)
    value = np.asarray(value, np.float32)
    query_mask = np.asarray(query_mask, bool)
    key_mask = np.asarray(key_mask, bool)
    Wq = np.asarray(Wq, np.float32); Wk = np.asarray(Wk, np.float32)
    bq = np.asarray(bq, np.float32); bk = np.asarray(bk, np.float32)
    gq = np.asarray(gq, np.float32); gk = np.asarray(gk, np.float32)
    betaq = np.asarray(betaq, np.float32); betak = np.asarray(betak, np.float32)

    Q, B, Hh = query.shape
    assert Hh == H and B == NCORES

    qidx = [np.nonzero(query_mask[:, b])[0] for b in range(B)]
    kidx = [np.nonzero(key_mask[:, b])[0] for b in range(B)]
    maxn = max([len(i) for i in qidx + kidx] + [1])
    pad = max(1152, -(-maxn // 128) * 128)

    biasq = bool(np.any(bq)); biask = bool(np.any(bk))
    affq = not (np.all(gq == 1.0) and not np.any(betaq))
    affk = not (np.all(gk == 1.0) and not np.any(betak))
    nc = _get_nc(pad, biasq, biask, affq, affk)

    wqT = np.ascontiguousarray(Wq.T).astype(ml_dtypes.bfloat16)
    wkT = np.ascontiguousarray(Wk.T).astype(ml_dtypes.bfloat16)
    in_maps = []
    for b in range(B):
        qi, ki = qidx[b], kidx[b]
        xq = np.zeros((pad, H), ml_dtypes.bfloat16)
        xq[:len(qi)] = query[qi, b].astype(ml_dtypes.bfloat16)
        xk = np.zeros((pad, H), ml_dtypes.bfloat16)
        xk[:len(ki)] = key_in[ki, b].astype(ml_dtypes.bfloat16)
        vv = np.zeros((pad, H), ml_dtypes.bfloat16)
        vv[:len(ki)] = value[ki, b].astype(ml_dtypes.bfloat16)
        m = {
            "xqT": np.ascontiguousarray(xq.T),
            "xkT": np.ascontiguousarray(xk.T),
            "v": vv,
            "npad": np.full((1, 1), pad - len(ki), np.float32),
            "WqT": wqT,
            "WkT": wkT,
        }
        if biask or affk:
            km01 = np.zeros((pad, 1), np.float32); km01[:len(ki)] = 1.0
            m["km01"] = km01
        if biasq: m["bq"] = bq.reshape(1, H)
        if biask: m["bk"] = bk.reshape(1, H)
        if affq: m["gq"] = gq.reshape(1, H); m["betaq"] = betaq.reshape(1, H)
        if affk: m["gk"] = gk.reshape(1, H); m["betak"] = betak.reshape(1, H)
        in_maps.append(m)

    res = run_bass_kernel_spmd(nc, in_maps, core_ids=list(range(NCORES)))
    global last_results
    last_results = res

    out = np.zeros((Q, B, H), np.float32)
    for b in range(B):
        qi = qidx[b]
        out[qi, b, :] = res.results[b]["out"][:len(qi)]
    return out


# revision 11
# speedup vs baseline: 1.2602x; 1.2602x over previous
"""Masked attention kernel for Trainium2, data-parallel over 8 NeuronCores.

Problem: out[q,b,:] = softmax-ish(LN(query Wq^T+bq) @ LN(key Wk^T+bk)^T / sqrt(H),
masked by query_mask & key_mask, with the reference's idiosyncratic
exp(s - 2*rowmax) / (sum + 0.001) normalization) @ value.

Key observations exploited:
 - The reference fills masked scores with the GLOBAL min before the row max.
   Every unmasked score >= global min, so the row max equals the max over
   unmasked entries whenever one exists; fully-masked rows output exactly 0.
   Hence zero cross-batch communication: B=8 batches map 1:1 onto 8 cores.
 - Masked-out query rows produce zero output rows; masked-out keys contribute
   nothing.  Both masks are ~50% dense, so each core computes attention only
   over compacted (host-gathered) rows, padded to a fixed size.
 - exp(s - 2m)/(sum + 0.001) == exp(s)/(sum' + 0.001*exp(2m)), and
   exp(2m) == (rowmax of exp(s))^2.  So we take exp with no shift at all
   (scaled scores are O(5), safely in range) and correct the denominator with
   0.001*emax^2 - npad (each padded key column contributes exactly exp(0)=1).
   This kills the entire PSUM row-max reduction pass.

Host side: compact/pad/transpose per batch (cheap numpy), run the SPMD NEFF,
scatter results back into the full [Q,B,H] output.
"""

import numpy as np
import ml_dtypes

import concourse.bacc as bacc
import concourse.bass as bass
import concourse.tile as tile
from concourse import mybir, masks
from concourse.bass_utils import run_bass_kernel_spmd

F32 = mybir.dt.float32
BF16 = mybir.dt.bfloat16
AX = mybir.AxisListType.X
AF = mybir.ActivationFunctionType
ALU = mybir.AluOpType

H = 512
HC = H // 128          # contraction chunks over the hidden dim
NCORES = 8
RSQRT_H = 1.0 / float(np.sqrt(np.float32(H)))
EPS = 1e-5

_cache = {}
last_results = None


def _s_chunks(pad):
    """Split the key axis into <=512-wide chunks, one PSUM bank slot each."""
    nb = -(-pad // 512)
    base, rem = divmod(pad, nb)
    return [base + (1 if i < rem else 0) for i in range(nb)]


def _build(pad, biasq, biask, affq, affk):
    nt = pad // 128
    sch = _s_chunks(pad)
    nsb = len(sch)

    nc = bacc.Bacc(None, target_bir_lowering=False, debug=False, enable_asserts=False)

    xqT_d = nc.declare_dram_parameter("xqT", [H, pad], BF16, isOutput=False)
    xkT_d = nc.declare_dram_parameter("xkT", [H, pad], BF16, isOutput=False)
    v_d = nc.declare_dram_parameter("v", [pad, H], BF16, isOutput=False)
    npad_d = nc.declare_dram_parameter("npad", [1, 1], F32, isOutput=False)
    km_d = None
    if biask or affk:
        km_d = nc.declare_dram_parameter("km01", [pad, 1], F32, isOutput=False)
    wqT_d = nc.declare_dram_parameter("WqT", [H, H], BF16, isOutput=False)
    wkT_d = nc.declare_dram_parameter("WkT", [H, H], BF16, isOutput=False)
    extras_d = {}
    if biasq:
        extras_d["bq"] = nc.declare_dram_parameter("bq", [1, H], F32, isOutput=False)
    if biask:
        extras_d["bk"] = nc.declare_dram_parameter("bk", [1, H], F32, isOutput=False)
    if affq:
        extras_d["gq"] = nc.declare_dram_parameter("gq", [1, H], F32, isOutput=False)
        extras_d["betaq"] = nc.declare_dram_parameter("betaq", [1, H], F32, isOutput=False)
    if affk:
        extras_d["gk"] = nc.declare_dram_parameter("gk", [1, H], F32, isOutput=False)
        extras_d["betak"] = nc.declare_dram_parameter("betak", [1, H], F32, isOutput=False)
    out_d = nc.declare_dram_parameter("out", [pad, H], F32, isOutput=True)

    with tile.TileContext(nc) as tc:
        with (
            tc.tile_pool(name="persist", bufs=1) as persist,
            tc.tile_pool(name="small", bufs=6) as small,
            tc.tile_pool(name="lnt", bufs=4) as lnt,
            tc.tile_pool(name="ework", bufs=2) as ework,
            tc.tile_pool(name="ptw", bufs=3) as ptw,
            tc.tile_pool(name="osb", bufs=2) as osbp,
        ):
            ident16 = persist.tile([128, 128], BF16)
            masks.make_identity(nc, ident16[:])
            c001_t = persist.tile([128, 1], F32)
            nc.vector.memset(c001_t[:], 0.001)
            eps_t = persist.tile([128, 1], F32)
            nc.vector.memset(eps_t[:], EPS)

            # resident inputs; split across both HWDGE queues for parallel load
            xqT_sb = persist.tile([128, HC, pad], BF16)
            xq_r = xqT_d[:, :].rearrange("(c p) t -> p c t", p=128)
            nc.sync.dma_start(out=xqT_sb[:, 0:2, :], in_=xq_r[:, 0:2, :])
            nc.scalar.dma_start(out=xqT_sb[:, 2:4, :], in_=xq_r[:, 2:4, :])
            wq_sb = persist.tile([128, HC, H], BF16)
            nc.sync.dma_start(out=wq_sb[:], in_=wqT_d[:, :].rearrange("(c p) i -> p c i", p=128))
            xkT_sb = persist.tile([128, HC, pad], BF16)
            xk_r = xkT_d[:, :].rearrange("(c p) t -> p c t", p=128)
            nc.scalar.dma_start(out=xkT_sb[:, 0:2, :], in_=xk_r[:, 0:2, :])
            nc.sync.dma_start(out=xkT_sb[:, 2:4, :], in_=xk_r[:, 2:4, :])
            wk_sb = persist.tile([128, HC, H], BF16)
            nc.scalar.dma_start(out=wk_sb[:], in_=wkT_d[:, :].rearrange("(c p) i -> p c i", p=128))
            npad_t = persist.tile([128, 1], F32)
            np_src = npad_d[:, :]
            np_src = bass.AP(tensor=np_src.tensor, offset=np_src.offset, ap=[[0, 128], [1, 1]])
            nc.scalar.dma_start(out=npad_t[:], in_=np_src)
            kmA_sb = None
            if km_d is not None:
                kmA_sb = persist.tile([128, nt], F32)
                nc.scalar.dma_start(out=kmA_sb[:], in_=km_d[:, :].rearrange("(n p) o -> p (n o)", p=128))
            bcast = {}
            for name, dram in extras_d.items():
                t = persist.tile([128, H], F32, tag=f"bc_{name}")
                src = dram[:, :]
                src = bass.AP(tensor=src.tensor, offset=src.offset, ap=[[0, 128]] + [src.ap[-1]])
                nc.scalar.dma_start(out=t[:], in_=src)
                bcast[name] = t

            qT_sb = persist.tile([128, HC, pad], BF16)
            kT_sb = persist.tile([128, HC, pad], BF16)

            # ---- phase A: project + layernorm + transpose (SW-pipelined) --------
            with (
                tc.tile_pool(name="psA", bufs=5, space="PSUM") as psA,
                tc.tile_pool(name="psT", bufs=2, space="PSUM") as psT,
            ):
                jobs = [(s, t) for s in range(2) for t in range(nt)]
                ps_of = {}

                def proj(i):
                    s, t = jobs[i]
                    x_sb = (xqT_sb, xkT_sb)[s]
                    w_sb = (wq_sb, wk_sb)[s]
                    ps = psA.tile([128, H], F32, tag="proj")
                    ps_of[i] = ps
                    for c in range(HC):
                        nc.tensor.matmul(ps[:], x_sb[:, c, t * 128:(t + 1) * 128],
                                         w_sb[:, c, :], start=(c == 0), stop=(c == HC - 1))

                def ln_and_tp(i):
                    s, t = jobs[i]
                    dst = (qT_sb, kT_sb)[s]
                    use_bias = (biasq, biask)[s]
                    use_aff = (affq, affk)[s]
                    ps = ps_of.pop(i)
                    if use_bias:
                        nc.vector.tensor_add(ps[:], ps[:], bcast[("bq", "bk")[s]][:])
                    stats = small.tile([128, 6], F32, tag="stats")
                    nc.vector.bn_stats(out=stats[:], in_=ps[:])
                    mv = small.tile([128, 2], F32, tag="mv")
                    nc.vector.bn_aggr(out=mv[:], in_=stats[:])
                    sd = small.tile([128, 1], F32, tag="sd")
                    nc.scalar.activation(out=sd[:], in_=mv[:, 1:2], func=AF.Sqrt,
                                         bias=eps_t[:], scale=1.0)
                    rstd = small.tile([128, 1], F32, tag="rstd")
                    nc.vector.reciprocal(out=rstd[:], in_=sd[:])
                    nmr = small.tile([128, 1], F32, tag="nmr")
                    nc.gpsimd.tensor_mul(nmr[:], mv[:, 0:1], rstd[:])
                    nc.gpsimd.tensor_scalar_mul(nmr[:], nmr[:], -1.0)
                    ln = lnt.tile([128, H], BF16, tag="ln")
                    if use_aff:
                        ln32 = lnt.tile([128, H], F32, tag="ln32")
                        nc.scalar.activation(out=ln32[:], in_=ps[:], func=AF.Identity,
                                             bias=nmr[:], scale=rstd[:])
                        nc.vector.tensor_mul(ln32[:], ln32[:], bcast[("gq", "gk")[s]][:])
                        nc.vector.tensor_add(ln[:], ln32[:], bcast[("betaq", "betak")[s]][:])
                    elif i % 2 == 0:
                        nc.scalar.activation(out=ln[:], in_=ps[:], func=AF.Identity,
                                             bias=nmr[:], scale=rstd[:])
                    else:
                        nc.vector.tensor_scalar(out=ln[:], in0=ps[:], scalar1=rstd[:],
                                                scalar2=nmr[:], op0=ALU.mult, op1=ALU.add)
                    if s == 1 and kmA_sb is not None:
                        nc.vector.tensor_scalar_mul(ln[:], ln[:], kmA_sb[:, t:t + 1])
                    tp = psT.tile([128, HC, 128], BF16, tag="tpA")
                    for c in range(HC):
                        nc.tensor.transpose(tp[:, c, :], ln[:, c * 128:(c + 1) * 128], ident16[:])
                    dslice = dst[:, :, t * 128:(t + 1) * 128]
                    if i % 2 == 0:
                        nc.vector.tensor_copy(dslice, tp[:])
                    else:
                        nc.scalar.copy(dslice, tp[:])

                DEPTH = 4
                for i in range(min(DEPTH, len(jobs))):
                    proj(i)
                for i in range(len(jobs)):
                    if i + DEPTH < len(jobs):
                        proj(i + DEPTH)
                    ln_and_tp(i)

            # ---- phase B: attention (SW-pipelined) ------------------------------
            v_sb = persist.tile([128, nt, H], BF16)
            nc.sync.dma_start(out=v_sb[:], in_=v_d[:, :].rearrange("(n p) h -> p n h", p=128))
            with (
                tc.tile_pool(name="psS", bufs=2, space="PSUM") as psS,
                tc.tile_pool(name="psC", bufs=1, space="PSUM") as psC,
                tc.tile_pool(name="psP", bufs=1, space="PSUM") as psP,
            ):
                S_of = {}

                def qk(t):
                    S = psS.tile([128, nsb, 512], F32, tag="S")
                    S_of[t] = S
                    for c in range(HC):
                        off = 0
                        for j, w in enumerate(sch):
                            nc.tensor.matmul(S[:, j, 0:w],
                                             qT_sb[:, c, t * 128:(t + 1) * 128],
                                             kT_sb[:, c, off:off + w],
                                             start=(c == 0), stop=(c == HC - 1))
                            off += w

                def attend(t):
                    S = S_of.pop(t)
                    e = ework.tile([128, pad], BF16, tag="e")
                    acc3 = small.tile([128, nsb], F32, tag="acc3")
                    off = 0
                    for j, w in enumerate(sch):
                        nc.scalar.activation(out=e[:, off:off + w], in_=S[:, j, 0:w],
                                             func=AF.Exp, bias=0.0, scale=RSQRT_H,
                                             accum_out=acc3[:, j:j + 1])
                        off += w
                    # denom = sum(e) - npad + 0.001 * rowmax(e)^2
                    emax = small.tile([128, 1], F32, tag="emax")
                    nc.vector.reduce_max(emax[:], e[:], axis=AX)
                    dsum = small.tile([128, 1], F32, tag="dsum")
                    nc.vector.reduce_sum(dsum[:], acc3[:], axis=AX)
                    nc.gpsimd.tensor_mul(emax[:], emax[:], emax[:])
                    nc.gpsimd.tensor_scalar(out=emax[:], in0=emax[:], scalar1=0.001,
                                            scalar2=npad_t[:], op0=ALU.mult, op1=ALU.subtract)
                    nc.gpsimd.tensor_add(dsum[:], dsum[:], emax[:])
                    r = small.tile([128, 1], F32, tag="r")
                    nc.vector.reciprocal(out=r[:], in_=dsum[:])

                    C = psC.tile([128, H], F32, tag="C")
                    tpb = psP.tile([128, 8, 128], BF16, tag="tpb")
                    groups = [list(range(g * 4, min(g * 4 + 4, nt))) for g in range(-(-nt // 4))]
                    pt4s = []
                    for g, js in enumerate(groups):
                        base = (g % 2) * 4
                        for i, j in enumerate(js):
                            nc.tensor.transpose(tpb[:, base + i, :],
                                                e[:, j * 128:(j + 1) * 128], ident16[:])
                        pt4 = ptw.tile([128, 4, 128], BF16, tag="pt")
                        srcap = tpb[:, base:base + len(js), :]
                        if g % 2 == 0:
                            nc.vector.tensor_copy(pt4[:, 0:len(js), :], srcap)
                        else:
                            nc.scalar.copy(pt4[:, 0:len(js), :], srcap)
                        pt4s.append((pt4, js))
                        if g > 0:
                            pprev, pjs = pt4s[g - 1]
                            for i, j in enumerate(pjs):
                                nc.tensor.matmul(C[:], pprev[:, i, :], v_sb[:, j, :],
                                                 start=(j == 0), stop=(j == nt - 1))
                    pprev, pjs = pt4s[-1]
                    for i, j in enumerate(pjs):
                        nc.tensor.matmul(C[:], pprev[:, i, :], v_sb[:, j, :],
                                         start=(j == 0), stop=(j == nt - 1))
                    o = osbp.tile([128, H], F32, tag="o")
                    if t % 2 == 0:
                        nc.vector.tensor_scalar_mul(o[:], C[:], r[:])
                    else:
                        nc.scalar.activation(out=o[:], in_=C[:], func=AF.Copy, scale=r[:])
                    nc.sync.dma_start(out=out_d[t * 128:(t + 1) * 128, :], in_=o[:])

                qk(0)
                for t in range(nt):
                    if t + 1 < nt:
                        qk(t + 1)
                    attend(t)

    nc.compile()
    return nc


def _get_nc(pad, biasq, biask, affq, affk):
    key = (pad, biasq, biask, affq, affk)
    if key not in _cache:
        _cache[key] = _build(*key)
    return _cache[key]


def kernel(query, key_in, value, query_mask, key_mask,
           Wq, bq, gq, betaq, Wk, bk, gk, betak):
    query = np.asarray(query, np.float32)
    key_in = np.asarray(key_in, np.float32)
    value = np.asarray(value, np.float32)
    query_mask = np.asarray(query_mask, bool)
    key_mask = np.asarray(key_mask, bool)
    Wq = np.asarray(Wq, np.float32); Wk = np.asarray(Wk, np.float32)
    bq = np.asarray(bq, np.float32); bk = np.asarray(bk, np.float32)
    gq = np.asarray(gq, np.float32); gk = np.asarray(gk, np.float32)
    betaq = np.asarray(betaq, np.float32); betak = np.asarray(betak, np.float32)

    Q, B, Hh = query.shape
    assert Hh == H and B == NCORES

    qidx = [np.nonzero(query_mask[:, b])[0] for b in range(B)]
    kidx = [np.nonzero(key_mask[:, b])[0] for b in range(B)]
    maxn = max([len(i) for i in qidx + kidx] + [1])
    pad = max(1152, -(-maxn // 128) * 128)

    biasq = bool(np.any(bq)); biask = bool(np.any(bk))
    affq = not (np.all(gq == 1.0) and not np.any(betaq))
    affk = not (np.all(gk == 1.0) and not np.any(betak))
    nc = _get_nc(pad, biasq, biask, affq, affk)

    wqT = np.ascontiguousarray(Wq.T).astype(ml_dtypes.bfloat16)
    wkT = np.ascontiguousarray(Wk.T).astype(ml_dtypes.bfloat16)
    in_maps = []
    for b in range(B):
        qi, ki = qidx[b], kidx[b]
        xq = np.zeros((pad, H), ml_dtypes.bfloat16)
        xq[:len(qi)] = query[qi, b].astype(ml_dtypes.bfloat16)
        xk = np.zeros((pad, H), ml_dtypes.bfloat16)
        xk[:len(ki)] = key_in[ki, b].astype(ml_dtypes.bfloat16)
        vv = np.zeros((pad, H), ml_dtypes.bfloat16)
        vv[:len(ki)] = value[ki, b].astype(ml_dtypes.bfloat16)
        m = {
            "xqT": np.ascontiguousarray(xq.T),
            "xkT": np.ascontiguousarray(xk.T),
            "v": vv,
            "npad": np.full((1, 1), pad - len(ki), np.float32),
            "WqT": wqT,
            "WkT": wkT,
        }
        if biask or affk:
            km01 = np.zeros((pad, 1), np.float32); km01[:len(ki)] = 1.0
            m["km01"] = km01
        if biasq: m["bq"] = bq.reshape(1, H)
        if biask: m["bk"] = bk.reshape(1, H)
        if affq: m["gq"] = gq.reshape(1, H); m["betaq"] = betaq.reshape(1, H)
        if affk: m["gk"] = gk.reshape(1, H); m["betak"] = betak.reshape(1, H)
        in_maps.append(m)

    res = run_bass_kernel_spmd(nc, in_maps, core_ids=list(range(NCORES)))
    global last_results
    last_results = res

    out = np.zeros((Q, B, H), np.float32)
    for b in range(B):
        qi = qidx[b]
        out[qi, b, :] = res.results[b]["out"][:len(qi)]
    return out
